# revision 3
# baseline (speedup 1.0000x reference)
"""Trainium2 Bass kernel for CovarianceComplexBatchNorm (training-mode complex BN).

Contract: kernel(**inputs) takes the FULL unsharded inputs
  real [65536, 1024] f32, imag [65536, 1024] f32,
  gamma_rr/gamma_ri/gamma_ii/beta_real/beta_imag [1024] f32
and returns (out_r, out_i), both [65536, 1024] f32 — matching reference.py.

Strategy: data-parallel over the batch dim across 8 NeuronCores.
  Pass A: per-core partial sums of (r, i, r^2, i^2, r*i) per feature via
          TensorE ones-matmul reduction into PSUM.
  AllReduce of the 5x1024 stats vector.
  Coef stage: whitening math on a feature-on-partition [128, 8] layout
          (PE transposes in/out), producing a fused affine transform
          out_r = a_rr*r + a_ri*i + b_r ; out_i = a_ir*r + a_ii*i + b_i
          with the means folded into b_*.
  Pass B: streaming application of the affine transform.
"""

from contextlib import ExitStack

import numpy as np

import concourse.bacc as bacc
import concourse.bass as bass
import concourse.tile as tile
from concourse import mybir
from concourse.bass_utils import run_bass_kernel_spmd

F32 = mybir.dt.float32
EPS = 1e-5

# Full-problem constants (hardcoded per harness contract).
N_FULL = 65536
F_FULL = 1024
N_CORES = 8
P = 128


def _row_bcast(ap_row, parts):
    """AP view replicating a [1, F] row across `parts` partitions (step-0)."""
    return bass.AP(
        tensor=ap_row.tensor,
        offset=ap_row.offset,
        ap=[[0, parts]] + [list(d) for d in ap_row.ap[1:]],
    )


def build_kernel(nl, n_total, n_cores, f=F_FULL):
    """Builds + compiles the per-core Bass program. Returns the nc object."""
    nc = bacc.Bacc(
        "TRN2",
        target_bir_lowering=False,
        debug=False,
        enable_asserts=False,
        num_devices=n_cores,
    )

    real = nc.dram_tensor("real", [nl, f], F32, kind="ExternalInput")
    imag = nc.dram_tensor("imag", [nl, f], F32, kind="ExternalInput")
    params = {
        name: nc.dram_tensor(name, [1, f], F32, kind="ExternalInput")
        for name in ["gamma_rr", "gamma_ri", "gamma_ii", "beta_real", "beta_imag"]
    }
    out_r = nc.dram_tensor("out_r", [nl, f], F32, kind="ExternalOutput")
    out_i = nc.dram_tensor("out_i", [nl, f], F32, kind="ExternalOutput")

    nt = nl // P
    nh = f // 512  # matmul moving-operand chunks (fp32 free-dim max 512)
    nchunk = f // P  # feature chunks of 128 for the transposed coef stage
    inv_n = 1.0 / float(n_total)

    with tile.TileContext(nc) as tc, ExitStack() as ctx:
        singles = ctx.enter_context(tc.tile_pool(name="singles", bufs=1))
        dram = ctx.enter_context(tc.tile_pool(name="dram", bufs=1, space="DRAM"))

        # --- constants ---------------------------------------------------
        # one-hot selector columns: sel[:, 0:4]=0, sel[:,4]=1, sel[:,5:9]=0
        # lhsT for stat s is sel[:, 4-s : 9-s]  ([128, 5], one-hot col s).
        sel = singles.tile([P, 9], F32)
        nc.vector.memset(sel, 0.0)
        nc.vector.memset(sel[:, 4:5], 1.0)

        # identity matrix (for PE transposes and identity-matmul adds)
        ones_sq = singles.tile([P, P], F32)
        nc.vector.memset(ones_sq, 1.0)
        idn = singles.tile([P, P], F32)
        nc.gpsimd.affine_select(
            out=idn,
            in_=ones_sq,
            pattern=[[1, P]],
            compare_op=mybir.AluOpType.is_equal,
            fill=0.0,
            base=0,
            channel_multiplier=-1,
        )

        # combo rows 0-4: allreduced stats (r, i, rr, ii, ri); rows 5-9 params
        combo = singles.tile([10, f], F32)
        for k, name in enumerate(
            ["gamma_rr", "gamma_ri", "gamma_ii", "beta_real", "beta_imag"]
        ):
            nc.sync.dma_start(combo[5 + k : 6 + k, :], params[name][:, :])

        # ============ Pass A: per-feature partial sums ====================
        with tc.tile_pool(name="pstats", bufs=1, space="PSUM") as pstats_pool, \
             tc.tile_pool(name="loadA", bufs=3) as loadA, \
             tc.tile_pool(name="sqA", bufs=2) as sqA:
            pstats = pstats_pool.tile([5, f], F32)
            for t in range(nt):
                rows = slice(t * P, (t + 1) * P)
                r_t = loadA.tile([P, f], F32, tag="r", name="r_t")
                i_t = loadA.tile([P, f], F32, tag="i", name="i_t")
                nc.sync.dma_start(r_t, real[rows, :])
                nc.sync.dma_start(i_t, imag[rows, :])
                rr_t = sqA.tile([P, f], F32, tag="rr", name="rr_t")
                ii_t = sqA.tile([P, f], F32, tag="ii", name="ii_t")
                ri_t = sqA.tile([P, f], F32, tag="ri", name="ri_t")
                nc.scalar.square(rr_t, r_t)
                nc.vector.tensor_mul(ii_t, i_t, i_t)
                nc.gpsimd.tensor_mul(ri_t, r_t, i_t)
                for s, src in enumerate([r_t, i_t, rr_t, ii_t, ri_t]):
                    lhsT = sel[:, 4 - s : 9 - s]
                    for h in range(nh):
                        cols = slice(h * 512, (h + 1) * 512)
                        nc.tensor.matmul(
                            pstats[:, cols],
                            lhsT,
                            src[:, cols],
                            start=(t == 0 and s == 0),
                            stop=(t == nt - 1 and s == 4),
                        )
            stats_sb = singles.tile([5, f], F32)
            nc.vector.tensor_copy(stats_sb, pstats)

        # ============ AllReduce of the 5 stat vectors =====================
        ar_in = dram.tile([5, f], F32)
        ar_out = dram.tile([5, f], F32)
        nc.sync.dma_start(ar_in, stats_sb)
        if n_cores > 1:
            nc.gpsimd.collective_compute(
                "AllReduce",
                mybir.AluOpType.add,
                replica_groups=[list(range(n_cores))],
                ins=[ar_in.opt()],
                outs=[ar_out.opt()],
            )
        else:
            # single-core build (cost-model runs): no collective needed
            nc.sync.dma_start(ar_out, ar_in)
        nc.sync.dma_start(combo[0:5, :], ar_out)

        # ============ Coefficient stage ===================================
        # Transpose combo [10, f] into vec [128, 10, f/128] (feature-major on
        # partitions) so all per-feature math runs 128-wide.
        bc = []  # six broadcast coefficient tiles [P, f]
        with tc.tile_pool(name="midp", bufs=1, space="PSUM") as midp, \
             tc.tile_pool(name="mid", bufs=1) as mid:
            psum_t = midp.tile([P, nchunk, 10], F32)
            for c in range(nchunk):
                nc.tensor.transpose(
                    psum_t[:, c, :],
                    combo[:, c * P : (c + 1) * P],
                    idn[0:10, 0:10],
                )
            vec = mid.tile([P, 10, nchunk], F32)
            nc.vector.tensor_copy(vec, psum_t.rearrange("p c s -> p s c"))

            def V(k):
                return vec[:, k, :]

            Sr, Si, Srr, Sii, Sri = (V(k) for k in range(5))
            Grr, Gri, Gii, Br, Bi = (V(k) for k in range(5, 10))

            def T(name):
                return mid.tile([P, nchunk], F32, name=name)

            alu = mybir.AluOpType
            stt = nc.vector.scalar_tensor_tensor

            mr = T("mr")
            mi = T("mi")
            nc.vector.tensor_scalar_mul(mr, Sr, inv_n)
            nc.vector.tensor_scalar_mul(mi, Si, inv_n)
            mrr = T("mrr")
            mii = T("mii")
            mri = T("mri")
            nc.vector.tensor_mul(mrr, mr, mr)
            nc.vector.tensor_mul(mii, mi, mi)
            nc.vector.tensor_mul(mri, mr, mi)
            # C_xx = S_xx/N - m_xx (+ EPS on the diagonal)
            crr = T("crr")
            cii = T("cii")
            cri = T("cri")
            stt(crr, Srr, inv_n, mrr, alu.mult, alu.subtract)
            nc.vector.tensor_scalar_add(crr, crr, EPS)
            stt(cii, Sii, inv_n, mii, alu.mult, alu.subtract)
            nc.vector.tensor_scalar_add(cii, cii, EPS)
            stt(cri, Sri, inv_n, mri, alu.mult, alu.subtract)
            # det = crr*cii - cri^2 ; s = sqrt(det)
            det = T("det")
            tmp0 = T("tmp0")
            nc.vector.tensor_mul(det, crr, cii)
            nc.vector.tensor_mul(tmp0, cri, cri)
            nc.vector.tensor_sub(det, det, tmp0)

            def sqrt_newton(out_name, x):
                """y = sqrt(x) via ACT sqrt + one Newton step (ACT sqrt has a
                loose ULP budget)."""
                y0 = T(out_name + "_y0")
                nc.scalar.sqrt(y0, x)
                rc = T(out_name + "_rc")
                nc.vector.reciprocal(rc, y0)
                h = T(out_name + "_h")
                nc.vector.tensor_mul(h, x, rc)
                y = T(out_name)
                nc.vector.tensor_add(y, y0, h)
                nc.vector.tensor_scalar_mul(y, y, 0.5)
                return y

            s_v = sqrt_newton("s_v", det)
            # t = sqrt(crr + cii + 2 s)
            tr = T("tr")
            nc.vector.tensor_add(tr, crr, cii)
            u2 = T("u2")
            stt(u2, s_v, 2.0, tr, alu.mult, alu.add)
            t_v = sqrt_newton("t_v", u2)
            den = T("den")
            nc.vector.tensor_mul(den, s_v, t_v)
            invd = T("invd")
            nc.vector.reciprocal(invd, den)
            # W = [[cii+s, -cri], [-cri, crr+s]] * invd
            wrr = T("wrr")
            wii = T("wii")
            wri = T("wri")
            nc.vector.tensor_add(wrr, cii, s_v)
            nc.vector.tensor_mul(wrr, wrr, invd)
            nc.vector.tensor_add(wii, crr, s_v)
            nc.vector.tensor_mul(wii, wii, invd)
            nc.vector.tensor_mul(wri, cri, invd)
            nc.vector.tensor_scalar_mul(wri, wri, -1.0)
            # fused affine coefficients (gamma is symmetric)
            cvec = mid.tile([P, 6, nchunk], F32)
            arr_ = cvec[:, 0, :]
            ari_ = cvec[:, 1, :]
            air_ = cvec[:, 2, :]
            aii_ = cvec[:, 3, :]
            br_ = cvec[:, 4, :]
            bi_ = cvec[:, 5, :]
            tmp1 = T("tmp1")
            nc.vector.tensor_mul(tmp1, Gri, wri)
            nc.vector.tensor_mul(arr_, Grr, wrr)
            nc.vector.tensor_add(arr_, arr_, tmp1)
            nc.vector.tensor_mul(tmp1, Gri, wii)
            nc.vector.tensor_mul(ari_, Grr, wri)
            nc.vector.tensor_add(ari_, ari_, tmp1)
            nc.vector.tensor_mul(tmp1, Gii, wri)
            nc.vector.tensor_mul(air_, Gri, wrr)
            nc.vector.tensor_add(air_, air_, tmp1)
            nc.vector.tensor_mul(tmp1, Gii, wii)
            nc.vector.tensor_mul(aii_, Gri, wri)
            nc.vector.tensor_add(aii_, aii_, tmp1)
            # b_r = Br - arr*mr - ari*mi ; b_i = Bi - air*mr - aii*mi
            nc.vector.tensor_mul(tmp1, arr_, mr)
            nc.vector.tensor_sub(br_, Br, tmp1)
            nc.vector.tensor_mul(tmp1, ari_, mi)
            nc.vector.tensor_sub(br_, br_, tmp1)
            nc.vector.tensor_mul(tmp1, air_, mr)
            nc.vector.tensor_sub(bi_, Bi, tmp1)
            nc.vector.tensor_mul(tmp1, aii_, mi)
            nc.vector.tensor_sub(bi_, bi_, tmp1)

            # transpose back: psum_ct[j, c, q] = cvec[q, j, c]
            psum_ct = midp.tile([6, nchunk, P], F32)
            for c in range(nchunk):
                nc.tensor.transpose(psum_ct[:, c, :], cvec[:, :, c], idn)
            coefT = mid.tile([6, nchunk, P], F32)
            nc.vector.tensor_copy(coefT, psum_ct)
            stage = dram.tile([6, f], F32)
            nc.sync.dma_start(stage, coefT)
            # broadcast each coefficient row across all 128 partitions
            for j in range(6):
                bc_j = singles.tile([P, f], F32, name=f"bc{j}", tag=f"bc{j}")
                nc.gpsimd.dma_start(bc_j, _row_bcast(stage[j : j + 1, :], P))
                bc.append(bc_j)

        bc_arr, bc_ari, bc_air, bc_aii, bc_br, bc_bi = bc

        # ============ Pass B: apply affine transform ======================
        with tc.tile_pool(name="loadB", bufs=3) as loadB, \
             tc.tile_pool(name="work", bufs=2) as work, \
             tc.tile_pool(name="outp", bufs=3) as outp, \
             tc.tile_pool(name="psumB", bufs=2, space="PSUM") as psumB:
            for t in range(nt):
                rows = slice(t * P, (t + 1) * P)
                r_t = loadB.tile([P, f], F32, tag="rB", name="r_t")
                i_t = loadB.tile([P, f], F32, tag="iB", name="i_t")
                nc.sync.dma_start(r_t, real[rows, :])
                nc.sync.dma_start(i_t, imag[rows, :])
                # out_r path on DVE
                u1 = work.tile([P, f], F32, tag="u1", name="u1")
                u2_ = work.tile([P, f], F32, tag="u2", name="u2_")
                nc.vector.tensor_mul(u1, r_t, bc_arr)
                nc.vector.tensor_mul(u2_, i_t, bc_ari)
                or_t = outp.tile([P, f], F32, tag="or", name="or_t")
                nc.vector.tensor_add(or_t, u1, u2_)
                nc.vector.tensor_add(or_t, or_t, bc_br)
                nc.scalar.dma_start(out_r[rows, :], or_t)
                # out_i path: products on GPSIMD, adds on PE via identity mm
                u3 = work.tile([P, f], F32, tag="u3", name="u3")
                u4 = work.tile([P, f], F32, tag="u4", name="u4")
                nc.gpsimd.tensor_mul(u3, r_t, bc_air)
                nc.gpsimd.tensor_mul(u4, i_t, bc_aii)
                psum_oi = psumB.tile([P, f], F32, tag="oi", name="psum_oi")
                for h in range(nh):
                    cols = slice(h * 512, (h + 1) * 512)
                    nc.tensor.matmul(
                        psum_oi[:, cols], idn, u3[:, cols], start=True, stop=False
                    )
                    nc.tensor.matmul(
                        psum_oi[:, cols], idn, u4[:, cols], start=False, stop=False
                    )
                    nc.tensor.matmul(
                        psum_oi[:, cols], idn, bc_bi[:, cols], start=False, stop=True
                    )
                oi_t = outp.tile([P, f], F32, tag="oi", name="oi_t")
                nc.scalar.copy(oi_t, psum_oi)
                nc.scalar.dma_start(out_i[rows, :], oi_t)

    nc.compile()
    return nc


_CACHE = {}


def _get_kernel(nl, n_total, n_cores, f):
    key = (nl, n_total, n_cores, f)
    if key not in _CACHE:
        _CACHE[key] = build_kernel(nl, n_total, n_cores, f)
    return _CACHE[key]


def kernel(real, imag, gamma_rr, gamma_ri, gamma_ii, beta_real, beta_imag,
           _trace=False):
    real = np.ascontiguousarray(np.asarray(real, dtype=np.float32))
    imag = np.ascontiguousarray(np.asarray(imag, dtype=np.float32))
    n, f = real.shape
    n_cores = N_CORES
    nl = n // n_cores
    params = {
        "gamma_rr": gamma_rr,
        "gamma_ri": gamma_ri,
        "gamma_ii": gamma_ii,
        "beta_real": beta_real,
        "beta_imag": beta_imag,
    }
    params = {
        k: np.ascontiguousarray(np.asarray(v, dtype=np.float32)).reshape(1, f)
        for k, v in params.items()
    }

    nc = _get_kernel(nl, n, n_cores, f)

    in_maps = []
    for c in range(n_cores):
        rows = slice(c * nl, (c + 1) * nl)
        in_map = {"real": real[rows], "imag": imag[rows]}
        in_map.update(params)
        in_maps.append(in_map)

    try:
        res = run_bass_kernel_spmd(
            nc, in_maps, core_ids=list(range(n_cores)), trace=_trace
        )
    except ModuleNotFoundError:
        # NTFF profiling hook unavailable in this environment
        res = run_bass_kernel_spmd(
            nc, in_maps, core_ids=list(range(n_cores)), trace=False
        )
    out_r = np.concatenate([res.results[c]["out_r"] for c in range(n_cores)], axis=0)
    out_i = np.concatenate([res.results[c]["out_i"] for c in range(n_cores)], axis=0)
    if _trace:
        kernel.last_results = res
    return out_r, out_i
